# revision 17
# baseline (speedup 1.0000x reference)
"""Bass/Trainium2 kernel for nn_PhysicsLoss (GNN message passing physics loss).

v3 strategy: shard 3.2M edges across 8 NeuronCores (400K each). Two window
layouts per core: A (partition p owns dst in [784p,784(p+1)), dst-sorted per
partition) and B (same for src). All data-dependent movement uses the only
fast per-partition-dynamic primitives on TRN2:
  - local_scatter (GPSIMD): per-partition 16-bit scatter at ~0.4 cyc/elem
  - PE transposes of [128,128] tiles for cross-partition exchange
A host-computed one-transpose Clos route (local_scatter -> 56 PE transposes
-> local_scatter) carries v[src] from the B layout (where it is
window-local) into A, and the per-edge currents from A into B. Per-node
segment sums are computed without any gather/scatter: prefix scan S, a
hold-scan P of S at run boundaries, D = S - P evaluated at run ends, placed
into the [128,784] node grid by one more local_scatter. node partial =
D - U; one 400KB AllReduce; mean(node_sum^2) + closed-form variance (KVL).
"""
import numpy as np

N_NODES = 100000
N_EDGES = 3200000
NCORES = 8
P = 128
W = 784                       # window nodes per partition; P*W = 100352
NPAD = P * W
EPC = N_EDGES // NCORES       # 400000
C = 3520                      # per-partition edge slots (incl. tail pads)
HC = C // 2                   # 1760: stage-3 / u16 chunk size
TMAX = 56
CR = TMAX * P                 # 7168 routing columns
CHK1 = CR // 4                # 1792: stage-1 dst chunk
EPS = 1e-6

_cache = {}
_last_in_maps = None


def _build(debug=False):
    import concourse.bass as bass
    import concourse.bacc as bacc
    import concourse.mybir as mybir
    from concourse.tile import TileContext
    from concourse.masks import make_identity

    f32 = mybir.dt.float32
    i16 = mybir.dt.int16
    u16 = mybir.dt.uint16
    bf16 = mybir.dt.bfloat16

    nc = bacc.Bacc("TRN2", target_bir_lowering=False, debug=False, num_devices=NCORES)

    v_d = nc.dram_tensor("v", [NPAD, 1], f32, kind="ExternalInput")
    logA_d = nc.dram_tensor("logA", [P, C], f32, kind="ExternalInput")
    parA_d = nc.dram_tensor("parA", [P, 2 * C], f32, kind="ExternalInput")
    stA_d = nc.dram_tensor("stA", [P, C], f32, kind="ExternalInput")
    stB_d = nc.dram_tensor("stB", [P, C], f32, kind="ExternalInput")
    ixVA_d = nc.dram_tensor("ixVA", [4, P, 2 * W], i16, kind="ExternalInput")
    ixVB_d = nc.dram_tensor("ixVB", [4, P, 2 * W], i16, kind="ExternalInput")
    ix1R_d = nc.dram_tensor("ix1R", [4, P, C], i16, kind="ExternalInput")
    ix3R_d = nc.dram_tensor("ix3R", [2, P, CR], i16, kind="ExternalInput")
    ix1S_d = nc.dram_tensor("ix1S", [4, P, C], i16, kind="ExternalInput")
    ix3S_d = nc.dram_tensor("ix3S", [2, P, CR], i16, kind="ExternalInput")
    ixD_d = nc.dram_tensor("ixD", [P, 2 * C], i16, kind="ExternalInput")
    ixU_d = nc.dram_tensor("ixU", [P, 2 * C], i16, kind="ExternalInput")
    out_d = nc.dram_tensor("out", [1, 1], f32, kind="ExternalOutput")
    dbg = {}
    if debug:
        for nm, sh in [("dvsA", [P, C]), ("dvdA", [P, C]), ("dcurA", [P, C]),
                       ("dDg", [P, W]), ("dcurB", [P, C]), ("dUg", [P, W]),
                       ("dns", [P, W])]:
            dbg[nm] = nc.dram_tensor(nm, sh, f32, kind="ExternalOutput")

    ns_d = nc.dram_tensor("ns", [NPAD, 1], f32)
    nsr_d = nc.dram_tensor("nsr", [NPAD, 1], f32)
    ns_2d = ns_d[:, :].rearrange("(p w) o -> p (w o)", p=P)
    nsr_2d = nsr_d[:, :].rearrange("(p w) o -> p (w o)", p=P)
    vgrid = v_d[:, :].rearrange("(p w) o -> p (w o)", p=P)

    ADD = mybir.AluOpType.add
    SUB = mybir.AluOpType.subtract
    MUL = mybir.AluOpType.mult
    MAX = mybir.AluOpType.max
    BYP = mybir.AluOpType.bypass

    with TileContext(nc) as tc:
        with (
            tc.tile_pool(name="pers", bufs=1) as pers,
            tc.tile_pool(name="big", bufs=1) as big,
            tc.tile_pool(name="bfp", bufs=1) as bfp,
            tc.tile_pool(name="ix", bufs=1) as ixp,
            tc.tile_pool(name="ps", bufs=2, space="PSUM") as ps,
        ):
            ones = pers.tile([P, 1], f32, tag="ones")
            nc.vector.memset(ones[:, :], 1.0)
            ksum = pers.tile([P, 4], f32, tag="ksum")
            kt4 = pers.tile([P, 4], f32, tag="kt4")
            identb = pers.tile([P, P], bf16, tag="identb")
            make_identity(nc, identb[:, :])
            VW = pers.tile([P, W], f32, tag="VW")
            nc.sync.dma_start(out=VW[:, :], in_=vgrid)
            Dg = pers.tile([P, W], f32, tag="Dg")
            Ug = pers.tile([P, W], f32, tag="Ug")

            F = {k: big.tile([P, C], f32, tag=f"F{k}", name=f"F{k}") for k in range(6)}
            B1 = bfp.tile([P, C], bf16, tag="B1")
            B2 = bfp.tile([P, C], bf16, tag="B2")
            R1 = bfp.tile([P, CR], bf16, tag="R1")
            R1T = bfp.tile([P, CR], bf16, tag="R1T")

            def ls(out_ap, data_ap, idx_ap, num_elems, num_idxs):
                nc.gpsimd.local_scatter(
                    out_ap=out_ap, data_ap=data_ap, idxs_ap=idx_ap,
                    channels=P, num_elems=num_elems, num_idxs=num_idxs,
                )

            def route(src_bf, dst_bf, ix1_d, ix3_d):
                for j in range(4):
                    ixt = ixp.tile([P, CR], i16, tag="ixb", name="ixb")
                    nc.sync.dma_start(out=ixt[:, :C], in_=ix1_d[j])
                    ls(R1[:, j * CHK1:(j + 1) * CHK1], src_bf[:, :], ixt[:, :C],
                       CHK1, C)
                for t in range(TMAX):
                    sl = bass.ds(t * P, P)
                    psT = ps.tile([P, P], bf16, tag="psT")
                    nc.tensor.transpose(
                        out=psT[:, :], in_=R1[:, sl], identity=identb[:, :]
                    )
                    nc.vector.tensor_copy(R1T[:, sl], psT[:, :])
                for k in range(2):
                    ixt = ixp.tile([P, CR], i16, tag="ixb", name="ixb")
                    nc.sync.dma_start(out=ixt[:, :], in_=ix3_d[k])
                    ls(dst_bf[:, k * HC:(k + 1) * HC], R1T[:, :], ixt[:, :],
                       HC, CR)

            def expand(ixV_d, start_t, sv_t, out_t):
                # place VW values at run starts (u16 halves, exact), hold-scan
                for j in range(4):
                    ixt = ixp.tile([P, CR], i16, tag="ixb", name="ixb")
                    nc.sync.dma_start(out=ixt[:, :2 * W], in_=ixV_d[j])
                    ls(sv_t[:, j * (HC // 2):(j + 1) * (HC // 2)].bitcast(u16),
                       VW[:, :].bitcast(u16), ixt[:, :2 * W], HC, 2 * W)
                # keep = 1 - start, then state = keep*state + startval
                nc.vector.tensor_scalar(
                    out=out_t[:, :], in0=start_t[:, :], scalar1=-1.0, scalar2=1.0,
                    op0=MUL, op1=ADD,
                )
                nc.vector.tensor_tensor_scan(
                    out=out_t[:, :], data0=out_t[:, :], data1=sv_t[:, :],
                    initial=0.0, op0=MUL, op1=ADD,
                )

            def seg_diff(cur_t, start_t, S_t, SL_t, PA_t):
                # S = cumsum(cur); SL = S shifted right; P = hold(start*SL);
                # PA_t <- S - P (segment sum at each run's end col).
                # start_t is overwritten with keep = 1 - start.
                nc.vector.tensor_tensor_scan(
                    out=S_t[:, :], data0=cur_t[:, :], data1=cur_t[:, :],
                    initial=0.0, op0=ADD, op1=BYP,
                )
                nc.vector.memset(SL_t[:, 0:1], 0.0)
                nc.vector.tensor_copy(SL_t[:, 1:C], S_t[:, 0:C - 1])
                nc.vector.tensor_tensor(
                    out=SL_t[:, :], in0=start_t[:, :], in1=SL_t[:, :], op=MUL
                )
                nc.vector.tensor_scalar(
                    out=start_t[:, :], in0=start_t[:, :], scalar1=-1.0, scalar2=1.0,
                    op0=MUL, op1=ADD,
                )
                nc.vector.tensor_tensor_scan(
                    out=PA_t[:, :], data0=start_t[:, :], data1=SL_t[:, :],
                    initial=0.0, op0=MUL, op1=ADD,
                )
                nc.vector.tensor_tensor(
                    out=PA_t[:, :], in0=S_t[:, :], in1=PA_t[:, :], op=SUB
                )

            # ---------------- phase 1: A-side weights + KVL ---------------
            with tc.tile_pool(name="parp", bufs=1) as parp:
                part = parp.tile([P, 2 * C], f32, tag="part")
                nc.sync.dma_start(out=part[:, :], in_=parA_d[:, :])
                logt = F[0]
                nc.sync.dma_start(out=logt[:, :], in_=logA_d[:, :])
                par3 = part[:, :].rearrange("p (c two) -> p c two", two=2)
                imp = F[1]
                nc.vector.tensor_tensor(
                    out=imp[:, :], in0=par3[:, :, 0], in1=par3[:, :, 1], op=ADD
                )
                nc.vector.tensor_scalar_add(imp[:, :], imp[:, :], EPS)
                rec = F[2]
                nc.vector.reciprocal(rec[:, :], imp[:, :])
                sig = F[3]
                nc.scalar.activation(
                    sig[:, :], logt[:, :], mybir.ActivationFunctionType.Sigmoid
                )
                wA = F[4]
                nc.vector.tensor_tensor(out=wA[:, :], in0=sig[:, :], in1=rec[:, :], op=MUL)
                for half in range(2):
                    hs = bass.ds(half * C, C)
                    dst = ksum if half == 0 else kt4
                    sqh = F[5]
                    nc.vector.tensor_tensor(
                        out=sqh[:, :], in0=part[:, hs], in1=part[:, hs], op=MUL
                    )
                    ph3 = part[:, hs].rearrange("p (c two) -> p c two", two=2)
                    sh3 = sqh[:, :].rearrange("p (c two) -> p c two", two=2)
                    nc.vector.tensor_reduce(out=dst[:, 0:1], in_=ph3[:, :, 0], axis=mybir.AxisListType.X, op=ADD)
                    nc.vector.tensor_reduce(out=dst[:, 1:2], in_=sh3[:, :, 0], axis=mybir.AxisListType.X, op=ADD)
                    nc.vector.tensor_reduce(out=dst[:, 2:3], in_=ph3[:, :, 1], axis=mybir.AxisListType.X, op=ADD)
                    nc.vector.tensor_reduce(out=dst[:, 3:4], in_=sh3[:, :, 1], axis=mybir.AxisListType.X, op=ADD)
                nc.vector.tensor_tensor(out=ksum[:, :], in0=ksum[:, :], in1=kt4[:, :], op=ADD)

            # ---------------- phase 2: vsB expansion + route R ------------
            stB = F[0]
            nc.sync.dma_start(out=stB[:, :], in_=stB_d[:, :])
            SV = F[1]
            vsB = F[2]
            expand(ixVB_d, stB, SV, vsB)
            nc.vector.tensor_copy(B1[:, :], vsB[:, :])      # cast bf16
            route(B1, B2, ix1R_d, ix3R_d)                    # B2 = vsA (bf16)

            # ---------------- phase 3: A currents -------------------------
            stA = F[1]                                       # SV dead
            nc.sync.dma_start(out=stA[:, :], in_=stA_d[:, :])
            SVA = F[2]                                       # vsB dead
            vdA = F[3]                                       # sig dead
            expand(ixVA_d, stA, SVA, vdA)
            vsAf = F[5]                                      # sq-half dead
            nc.vector.tensor_copy(vsAf[:, :], B2[:, :])      # bf16 -> f32
            if debug:
                nc.sync.dma_start(out=dbg["dvsA"][:, :], in_=vsAf[:, :])
                nc.sync.dma_start(out=dbg["dvdA"][:, :], in_=vdA[:, :])
            dv = F[2]                                        # SVA dead
            nc.vector.tensor_tensor(out=dv[:, :], in0=vsAf[:, :], in1=vdA[:, :], op=SUB)
            adv = F[3]                                       # vdA dead
            nc.scalar.activation(
                adv[:, :], dv[:, :], mybir.ActivationFunctionType.Abs
            )
            curA = F[5]                                      # vsAf dead
            nc.vector.tensor_tensor(out=curA[:, :], in0=adv[:, :], in1=wA[:, :], op=MUL)
            if debug:
                nc.sync.dma_start(out=dbg["dcurA"][:, :], in_=curA[:, :])

            # ---------------- phase 4: D segment sums ---------------------
            SA, SLA, PA = F[2], F[3], F[4]                   # dv/ndv/wA dead
            seg_diff(curA, stA, SA, SLA, PA)
            ixtD = ixp.tile([P, CR], i16, tag="ixb", name="ixb")
            nc.sync.dma_start(out=ixtD[:, :2 * C], in_=ixD_d[:, :])
            ls(Dg[:, :].bitcast(u16), PA[:, :].bitcast(u16), ixtD[:, :2 * C],
               2 * W, 2 * C)
            if debug:
                nc.sync.dma_start(out=dbg["dDg"][:, :], in_=Dg[:, :])

            # ---------------- phase 5: route S + U segment sums -----------
            nc.vector.tensor_copy(B1[:, :], curA[:, :])      # cast bf16
            route(B1, B2, ix1S_d, ix3S_d)                    # B2 = curB (bf16)
            curB = F[2]                                      # SA dead
            nc.vector.tensor_copy(curB[:, :], B2[:, :])
            if debug:
                nc.sync.dma_start(out=dbg["dcurB"][:, :], in_=curB[:, :])
            SB, SLB, PB = F[3], F[4], F[5]                   # curA dead after cast
            seg_diff(curB, stB, SB, SLB, PB)
            ixtU = ixp.tile([P, CR], i16, tag="ixb", name="ixb")
            nc.sync.dma_start(out=ixtU[:, :2 * C], in_=ixU_d[:, :])
            ls(Ug[:, :].bitcast(u16), PB[:, :].bitcast(u16), ixtU[:, :2 * C],
               2 * W, 2 * C)
            if debug:
                nc.sync.dma_start(out=dbg["dUg"][:, :], in_=Ug[:, :])

            # ---------------- node sums + KVL stash + AllReduce -----------
            nst = pers.tile([P, W], f32, tag="nst")
            nc.vector.tensor_tensor(out=nst[:, :], in0=Dg[:, :], in1=Ug[:, :], op=SUB)
            if debug:
                nc.sync.dma_start(out=dbg["dns"][:, :], in_=nst[:, :])
            nc.sync.dma_start(out=ns_2d, in_=nst[:, :])

            kps = ps.tile([1, 4], f32, tag="kps")
            nc.tensor.matmul(kps[:, :], lhsT=ones[:, :], rhs=ksum[:, :], start=True, stop=True)
            prt4 = pers.tile([1, 4], f32, tag="prt4")
            nc.vector.tensor_copy(prt4[:, :], kps[:, :])
            nc.vector.tensor_scalar_mul(prt4[:, :], prt4[:, :], 2.0 ** -24)
            nc.sync.dma_start(
                out=ns_d[N_NODES:N_NODES + 4, :].rearrange("a o -> o a"),
                in_=prt4[:, :],
            )

            nc.gpsimd.collective_compute(
                "AllReduce",
                ADD,
                replica_groups=[list(range(NCORES))],
                ins=[ns_d.ap().opt()],
                outs=[nsr_d.ap().opt()],
            )

            # ---------------- final loss ----------------------------------
            red = pers.tile([P, W], f32, tag="red")
            nc.sync.dma_start(out=red[:, :], in_=nsr_2d)
            prt = pers.tile([1, 4], f32, tag="prt")
            nc.sync.dma_start(
                out=prt[:, :],
                in_=nsr_d[N_NODES:N_NODES + 4, :].rearrange("a o -> o a"),
            )
            ns2 = pers.tile([P, W], f32, tag="ns2")
            nc.vector.tensor_tensor(out=ns2[:, :], in0=red[:, :], in1=red[:, :], op=MUL)
            r1 = pers.tile([P, 1], f32, tag="r1")
            nc.vector.tensor_reduce(
                out=r1[:, :], in_=ns2[:, :], axis=mybir.AxisListType.X, op=ADD
            )
            kclp = ps.tile([1, 1], f32, tag="kclp")
            nc.tensor.matmul(kclp[:, :], lhsT=ones[:, :], rhs=r1[:, :], start=True, stop=True)
            kcl = pers.tile([1, 1], f32, tag="kcl")
            nc.vector.tensor_scalar_mul(kcl[:, :], kclp[:, :], 1.0 / N_NODES)
            nc.vector.tensor_scalar_mul(prt[:, :], prt[:, :], 2.0 ** 24)

            E = float(N_EDGES)
            pr3 = prt[:, :].rearrange("o (a b) -> o a b", b=2)
            s1 = pr3[:, 0:2, 0]
            s2 = pr3[:, 0:2, 1]
            m = pers.tile([1, 2], f32, tag="m")
            nc.vector.tensor_tensor(out=m[:, :], in0=s1, in1=s1, op=MUL)
            nc.vector.tensor_scalar_mul(m[:, :], m[:, :], -1.0 / E)
            nc.vector.tensor_tensor(out=m[:, :], in0=m[:, :], in1=s2, op=ADD)
            kvl = pers.tile([1, 1], f32, tag="kvl")
            nc.vector.tensor_reduce(
                out=kvl[:, :], in_=m[:, :], axis=mybir.AxisListType.X, op=ADD
            )
            nc.vector.tensor_scalar_mul(kvl[:, :], kvl[:, :], 0.5 / (E - 1.0))

            res = pers.tile([1, 1], f32, tag="res")
            nc.vector.tensor_tensor(out=res[:, :], in0=kcl[:, :], in1=kvl[:, :], op=ADD)
            nc.sync.dma_start(out=out_d[:, :], in_=res[:, :])

    nc.compile()
    return nc


def _layout(key):
    """Sort EPC edges by key into per-partition windows (partition = key//W,
    sorted by key within).  Returns placement info and run tables."""
    ord_ = np.argsort(key, kind="stable")
    ks = key[ord_]
    p = ks // W
    cnt = np.bincount(p, minlength=P)
    assert cnt.max() <= C - 1, cnt.max()
    starts = np.concatenate(([0], np.cumsum(cnt)[:-1]))
    col = np.arange(EPC, dtype=np.int64) - np.repeat(starts, cnt)
    first = np.ones(EPC, bool)
    first[1:] = ks[1:] != ks[:-1]
    st = np.zeros((P, C), np.float32)
    st[p, col] = first
    st[np.arange(P), cnt] = 1.0          # terminal marker
    runs = np.flatnonzero(first)
    run_p = p[runs]
    run_h = ks[runs] - W * run_p
    startcol = col[runs]
    endpos = np.empty(len(runs), np.int64)
    endpos[:-1] = runs[1:] - 1
    endpos[-1] = EPC - 1
    endcol = col[endpos]
    pe = np.empty(EPC, np.int64)
    ce = np.empty(EPC, np.int64)
    pe[ord_] = p
    ce[ord_] = col
    return ord_, p, col, pe, ce, st, run_p, run_h, startcol, endcol


def _expand_tables(run_p, run_h, startcol):
    """ixV[4, P, 2W] i16: VW u16 pair (2h,2h+1) -> u16 col 2*startcol (+b)."""
    ixV = np.full((4, P, 2 * W), -1, np.int16)
    tgt = 2 * startcol
    chunk = tgt // HC
    off = tgt - chunk * HC
    for b in (0, 1):
        ixV[chunk, run_p, 2 * run_h + b] = (off + b).astype(np.int16)
    return ixV


def _place_tables(run_p, run_h, endcol):
    """ixD [P, 2C] i16: source u16 col 2*endcol+b -> node grid u16 col 2h+b."""
    ixD = np.full((P, 2 * C), -1, np.int16)
    for b in (0, 1):
        ixD[run_p, 2 * endcol + b] = (2 * run_h + b).astype(np.int16)
    return ixD


def _pack_inputs(node_features, edge_index, edge_logits, edge_params):
    v = np.zeros((NPAD, 1), np.float32)
    v[:N_NODES, 0] = np.asarray(node_features[:, 0], np.float32)
    src = np.asarray(edge_index[0], dtype=np.int64)
    dst = np.asarray(edge_index[1], dtype=np.int64)
    logits = np.asarray(edge_logits, np.float32)
    params = np.asarray(edge_params, np.float32)

    in_maps = []
    for k in range(NCORES):
        sl = slice(k * EPC, (k + 1) * EPC)
        s, d, lg, pr = src[sl], dst[sl], logits[sl], params[sl]

        (ordA, pA, colA, pAe, cAe, stA, runAp, runAh, stcA, endA) = _layout(d)
        (ordB, pB, colB, pBe, cBe, stB, runBp, runBh, stcB, endB) = _layout(s)

        logA = np.zeros((P, C), np.float32)
        parA = np.zeros((P, C, 2), np.float32)
        logA[pA, colA] = lg[ordA]
        parA[pA, colA] = pr[ordA]

        ixVA = _expand_tables(runAp, runAh, stcA)
        ixVB = _expand_tables(runBp, runBh, stcB)
        ixD = _place_tables(runAp, runAh, endA)
        ixU = _place_tables(runBp, runBh, endB)

        # routing: t = rank of edge within its (pB, pA) partition-pair cell
        cell = pBe * P + pAe
        ordc = np.argsort(cell, kind="stable")
        cs = cell[ordc]
        cfirst = np.ones(EPC, bool)
        cfirst[1:] = cs[1:] != cs[:-1]
        gid = np.cumsum(cfirst) - 1
        gstart = np.flatnonzero(cfirst)
        t_sorted = np.arange(EPC) - gstart[gid]
        t = np.empty(EPC, np.int64)
        t[ordc] = t_sorted
        assert t.max() < TMAX, t.max()

        rcR = t * P + pAe          # stage-1 target col (route R, in part pBe)
        rcS = t * P + pBe          # stage-1 target col (route S, in part pAe)

        ix1R = np.full((4, P, C), -1, np.int16)
        ch = rcR // CHK1
        ix1R[ch, pBe, cBe] = (rcR - ch * CHK1).astype(np.int16)
        ix3R = np.full((2, P, CR), -1, np.int16)
        ch3 = cAe // HC
        ix3R[ch3, pAe, rcS] = (cAe - ch3 * HC).astype(np.int16)
        ix1S = np.full((4, P, C), -1, np.int16)
        chS = rcS // CHK1
        ix1S[chS, pAe, cAe] = (rcS - chS * CHK1).astype(np.int16)
        ix3S = np.full((2, P, CR), -1, np.int16)
        ch3S = cBe // HC
        ix3S[ch3S, pBe, rcR] = (cBe - ch3S * HC).astype(np.int16)

        in_maps.append({
            "v": v,
            "logA": logA,
            "parA": np.ascontiguousarray(parA.reshape(P, 2 * C)),
            "stA": stA, "stB": stB,
            "ixVA": ixVA, "ixVB": ixVB,
            "ix1R": ix1R, "ix3R": ix3R,
            "ix1S": ix1S, "ix3S": ix3S,
            "ixD": ixD, "ixU": ixU,
        })
    return in_maps


def kernel(node_features, edge_index, edge_logits, edge_params):
    global _last_in_maps
    from concourse.bass_utils import run_bass_kernel_spmd

    if "nc" not in _cache:
        _cache["nc"] = _build()
    nc = _cache["nc"]

    in_maps = _pack_inputs(node_features, edge_index, edge_logits, edge_params)
    _last_in_maps = in_maps
    res = run_bass_kernel_spmd(nc, in_maps, core_ids=list(range(NCORES)))
    return np.float32(res.results[0]["out"][0, 0])
